# revision 4
# baseline (speedup 1.0000x reference)
"""Causal self-attention (B=4, T=2048, C=1024, H=16) on 8 trn2 NeuronCores.

Sharding: core c -> batch b = c//2, heads h0 = (c%2)*8 .. h0+8 (tensor
parallel over heads: c_attn columns / c_proj rows split). Each core computes a
partial projection output [T, C] in bf16; the host sums the two partials per
batch and adds b_proj.

v2: single interleaved emission schedule so ScalarE exp overlaps all phases:
  - qkv projections chunked to [128,512] PSUM accumulators (1 bank each)
  - attention per (head, T/2-half): S^T superchunks [128,<=1024] -> exp ->
    PV accumulate into per-half yt [MV,1024] PSUM tile
  - v-stage / next-m qk-stage / proj chunks emitted as PE filler between
    attention chunks (keeps PE busy while exp runs, starts exp at ~25us)
  - output partials in bf16 (halves output DMA; rel-err ~4e-3 total)
"""

import numpy as np

P = 128


def _bf16_np():
    import ml_dtypes
    return ml_dtypes.bfloat16


def build_program(T=2048, C=1024, HC=8, D=64, num_devices=8, trn="TRN2"):
    import concourse.mybir as mybir
    import concourse.tile as tile
    from concourse import bacc
    from concourse.masks import make_upper_triangular

    W = 512          # matmul moving-dim chunk
    KC = C // P      # contraction tiles over C (8)
    CO = HC * D      # this core's qkv channel block (512)
    NP = CO // P     # head pairs per core (4)
    TT = T // P      # T tiles (16)
    T2 = T // 2      # half length (1024)
    NC512 = T // W   # 512-chunks across T (4)
    MV = 80          # PV stationary columns: D v-cols + 1 ones + pad
    dt32 = mybir.dt.float32
    f32r = mybir.dt.float32r
    bf16 = mybir.dt.bfloat16
    ActF = mybir.ActivationFunctionType
    Alu = mybir.AluOpType
    scale = 1.0 / float(np.sqrt(D))

    nc = bacc.Bacc(trn, target_bir_lowering=False, debug=False,
                   enable_asserts=False, num_devices=num_devices)

    xt_d = nc.dram_tensor("xt", [C, T], bf16, kind="ExternalInput")
    wq_d = nc.dram_tensor("wq", [C, CO], bf16, kind="ExternalInput")
    wk_d = nc.dram_tensor("wk", [C, CO], bf16, kind="ExternalInput")
    wv_d = nc.dram_tensor("wv", [C, CO], bf16, kind="ExternalInput")
    bq_d = nc.dram_tensor("bq", [P, NP], dt32, kind="ExternalInput")
    bk_d = nc.dram_tensor("bk", [P, NP], dt32, kind="ExternalInput")
    bvb_d = nc.dram_tensor("bvb", [P, CO], dt32, kind="ExternalInput")
    ones_d = nc.dram_tensor("ones", [P, TT * HC], bf16, kind="ExternalInput")
    wp_d = nc.dram_tensor("wp", [CO, C], f32r, kind="ExternalInput")
    out_d = nc.dram_tensor("out", [T, C], bf16, kind="ExternalOutput")
    lsc_d = nc.dram_tensor("lsc", [T], dt32)
    lsc2_d = nc.dram_tensor("lsc2", [T], dt32)

    with tile.TileContext(nc) as tc:
        with tc.tile_pool(name="const", bufs=1) as cpool, \
             tc.tile_pool(name="pers", bufs=1) as pers, \
             tc.tile_pool(name="wts", bufs=3 * KC) as wpool, \
             tc.tile_pool(name="wpj", bufs=1) as wppool, \
             tc.tile_pool(name="xtp", bufs=KC * NC512) as xpool, \
             tc.tile_pool(name="ptp", bufs=4) as ptpool, \
             tc.tile_pool(name="ysp", bufs=1) as yspool, \
             tc.tile_pool(name="nrm", bufs=1) as nrmpool, \
             tc.tile_pool(name="ost", bufs=3) as opool, \
             tc.tile_pool(name="psX", bufs=2, space="PSUM") as psX, \
             tc.tile_pool(name="psS", bufs=2, space="PSUM") as psS, \
             tc.tile_pool(name="psY", bufs=1, space="PSUM") as psY:

            # ---- constants, persistent tensors, prefetch DMAs ----
            tri = cpool.tile([P, P], bf16)
            make_upper_triangular(nc, tri[:], val=1.0, diag=True)
            bq_sb = cpool.tile([P, NP], dt32)
            nc.scalar.dma_start(bq_sb[:], bq_d.ap())
            bk_sb = cpool.tile([P, NP], dt32)
            nc.scalar.dma_start(bk_sb[:], bk_d.ap())
            bvb_sb = cpool.tile([P, CO], dt32)
            nc.scalar.dma_start(bvb_sb[:], bvb_d.ap())
            bvb_v = bvb_sb[:].rearrange("p (h d) -> p h d", d=D)

            qT = pers.tile([P, NP, T], bf16, tag="qT")
            kT = pers.tile([P, HC, T], bf16, tag="kT")
            vaug = pers.tile([P, TT, HC, MV], bf16, tag="vaug")
            yT = pers.tile([P, NP, T], f32r, tag="yT")
            nc.vector.memset(kT[:], 0.0)
            nc.vector.memset(vaug[:], 0.0)
            nc.scalar.dma_start(
                vaug[:, :, :, D],
                ones_d.ap().rearrange("p (a b) -> p a b", b=HC))

            xt_view = xt_d.ap().rearrange("(kc p) t -> kc p t", p=P)
            dmae = [nc.sync, nc.gpsimd]
            di = 0

            def dma(dst, src):
                nonlocal di
                dmae[di % 2].dma_start(dst, src)
                di += 1

            # first q-chunk needs wq[*] + xts[*][0]: those go first
            wq_t = []
            xts = [[None] * NC512 for _ in range(KC)]
            wq_view = wq_d.ap().rearrange("(kc p) n -> kc p n", p=P)
            for kc in range(KC):
                wt = wpool.tile([P, CO], bf16, tag="w")
                dma(wt[:], wq_view[kc])
                wq_t.append(wt)
                xtc = xpool.tile([P, W], bf16, tag="xt")
                dma(xtc[:], xt_view[kc][:, 0:W])
                xts[kc][0] = xtc
            for cg in range(1, NC512):
                for kc in range(KC):
                    xtc = xpool.tile([P, W], bf16, tag="xt")
                    dma(xtc[:], xt_view[kc][:, cg * W:(cg + 1) * W])
                    xts[kc][cg] = xtc

            def load_w(w_d):
                view = w_d.ap().rearrange("(kc p) n -> kc p n", p=P)
                tiles = []
                for kc in range(KC):
                    wt = wpool.tile([P, CO], bf16, tag="w")
                    dma(wt[:], view[kc])
                    tiles.append(wt)
                return tiles

            wk_t = load_w(wk_d)
            wv_t = load_w(wv_d)
            wpsb = wppool.tile([P, NP, C], f32r)

            # ---- stage emitters (each a small closure; order = schedule) ----
            def q_chunk(m, c):
                ps = psX.tile([P, W], dt32, tag="x")
                for kc in range(KC):
                    nc.tensor.matmul(
                        ps[:], wq_t[kc][:, m * P:(m + 1) * P], xts[kc][c][:],
                        start=(kc == 0), stop=(kc == KC - 1),
                        skip_group_check=True)
                nc.scalar.activation(
                    qT[:, m, c * W:(c + 1) * W], ps[:],
                    ActF.Identity, bias=bq_sb[:, m:m + 1], scale=1.0)

            def k_chunk(m, c):
                ps = psX.tile([P, W], dt32, tag="x")
                for kc in range(KC):
                    nc.tensor.matmul(
                        ps[:], wk_t[kc][:, m * P:(m + 1) * P], xts[kc][c][:],
                        start=(kc == 0), stop=(kc == KC - 1),
                        skip_group_check=True)
                sl = slice(c * W, (c + 1) * W)
                nc.scalar.activation(
                    kT[0:D, 2 * m, sl], ps[0:D, :],
                    ActF.Identity, bias=bk_sb[0:D, m:m + 1], scale=1.0)
                nc.scalar.activation(
                    kT[D:P, 2 * m + 1, sl], ps[D:P, :],
                    ActF.Identity, bias=bk_sb[D:P, m:m + 1], scale=1.0)

            def v_tt(tt):
                ps = psX.tile([P, CO], dt32, tag="x")
                c, off = (tt * P) // W, (tt * P) % W
                for kc in range(KC):
                    nc.tensor.matmul(
                        ps[:], xts[kc][c][:, off:off + P], wv_t[kc][:],
                        start=(kc == 0), stop=(kc == KC - 1),
                        skip_group_check=True)
                nc.vector.scalar_tensor_tensor(
                    out=vaug[:, tt, :, 0:D],
                    in0=ps[:].rearrange("p (h d) -> p h d", d=D),
                    scalar=1.0, in1=bvb_v,
                    op0=Alu.mult, op1=Alu.add)

            def s_chunk(h, half, j):
                m = h // 2
                jb = j * P
                lo, hi = half * T2, half * T2 + T2
                q0 = max(jb, lo)
                L = hi - q0
                pt = ptpool.tile([P, L], bf16, tag="pt")
                sps = psS.tile([P, T2], dt32, tag="s")
                for qs in range(q0, hi, W):
                    w = min(W, hi - qs)
                    nc.tensor.matmul(
                        sps[:, qs - q0:qs - q0 + w],
                        kT[:, h, jb:jb + P],
                        qT[:, m, qs:qs + w],
                        start=True, stop=True, skip_group_check=True)
                nc.scalar.activation(pt[:], sps[:, 0:L], ActF.Exp, scale=scale)
                if q0 == jb:
                    nc.vector.tensor_mul(pt[:, 0:P], pt[:, 0:P], tri[:])
                return pt

            def pv_chunk(h, half, j, pt, yt):
                # chunks aligned to the W grid so each matmul output stays
                # within one PSUM bank of yt
                jb = j * P
                lo, hi = half * T2, half * T2 + T2
                q0 = max(jb, lo)
                for cg in range(q0 // W, hi // W):
                    qs = max(W * cg, q0)
                    w = W * (cg + 1) - qs
                    last_j = (qs + w - 1) // P
                    nc.tensor.matmul(
                        yt[:, qs - lo:qs - lo + w],
                        vaug[:, j, h, :],
                        pt[:, qs - q0:qs - q0 + w],
                        start=(j == 0), stop=(j == last_j),
                        skip_group_check=True)

            def finish(h, half, yt):
                m, r0 = h // 2, (h % 2) * D
                lo = half * T2
                ys = yspool.tile([D + 1, T2], dt32, tag="ys")
                nc.vector.tensor_copy(ys[:], yt[0:D + 1, :])
                nc.sync.dma_start(
                    lsc_d.ap()[lo:lo + T2].rearrange("(o t) -> o t", o=1),
                    ys[D:D + 1, :])
                l128 = nrmpool.tile([P, T2 // P], dt32, tag="l128")
                nc.gpsimd.dma_start(
                    l128[:],
                    lsc_d.ap()[lo:lo + T2].rearrange("(p c) -> p c", p=P))
                nc.vector.reciprocal(l128[:], l128[:])
                nc.gpsimd.dma_start(
                    lsc2_d.ap()[lo:lo + T2].rearrange("(p c) -> p c", p=P),
                    l128[:])
                bc = nrmpool.tile([D, T2], dt32, tag="bc")
                nc.sync.dma_start(
                    bc[:],
                    lsc2_d.ap()[lo:lo + T2].rearrange(
                        "(o t) -> o t", o=1).broadcast_to([D, T2]))
                nc.vector.tensor_mul(
                    yT[r0:r0 + D, m, lo:lo + T2], ys[0:D, :], bc[:])

            def proj_chunk(tt, nn):
                po = psX.tile([P, W], dt32, tag="x")
                for kt in range(NP):
                    nc.tensor.matmul(
                        po[:], yT[:, kt, tt * P:(tt + 1) * P],
                        wpsb[:, kt, nn * W:(nn + 1) * W],
                        start=(kt == 0), stop=(kt == NP - 1),
                        skip_group_check=True)
                ot = opool.tile([P, W], bf16, tag="ot")
                nc.vector.tensor_copy(ot[:], po[:])
                dma(out_d.ap()[tt * P:(tt + 1) * P, nn * W:(nn + 1) * W],
                    ot[:])

            # ---- the schedule ----
            # Per head: 24 attention chunk-iterations (half0: j=0..7,
            # half1: j=0..15). A per-head filler stream is paced across
            # them: (wanted(i) - emitted) closures run before chunk i.
            def head_fillers(h):
                items, wanted = [], None
                if h == 0:
                    # v-stage; v(tt) must land before pv(h0,half0,j=tt)
                    items = [lambda tt=tt: v_tt(tt) for tt in range(TT)]
                    wanted = lambda i: i + 1 if i < 8 else 8 + (i - 7) // 2
                elif h in (1, 3, 5):
                    mm = h // 2 + 1
                    for c in range(NC512):
                        items.append(lambda c=c: q_chunk(mm, c))
                        items.append(lambda c=c: k_chunk(mm, c))
                    wanted = lambda i: (i * 8) // 24 + 1
                elif h == 6:
                    items = [lambda: nc.scalar.dma_start(
                        wpsb[:],
                        wp_d.ap().rearrange("(kt p) n -> p kt n", p=P))]
                    wanted = lambda i: 1
                elif h == 7:
                    # proj tt 0..7 ready after all half0 finishes (by h7
                    # half0 finish); pace them through h7 half1
                    items = [lambda t=t: proj_chunk(t // 2, t % 2)
                             for t in range(16)]
                    wanted = lambda i: max(0, i - 7)
                return items, wanted

            # initial qk for m=0
            for c in range(NC512):
                q_chunk(0, c)
                k_chunk(0, c)

            DEPTH = 2
            for h in range(HC):
                fillers, wanted = head_fillers(h)
                fi = 0
                i = 0  # chunk counter across both halves
                for half in (0, 1):
                    jhi = (half * T2 + T2) // P  # j in [0, jhi)
                    yt = psY.tile([MV, T2], dt32, tag="yt")
                    pend = []
                    for j in range(jhi):
                        # fillers first so PE has work while exp catches up
                        while fi < len(fillers) and fi < wanted(i):
                            fillers[fi]()
                            fi += 1
                        pt = s_chunk(h, half, j)
                        pend.append((j, pt))
                        if len(pend) > DEPTH:
                            pj, ppt = pend.pop(0)
                            pv_chunk(h, half, pj, ppt, yt)
                        i += 1
                    if half == 1:
                        while fi < len(fillers):
                            fillers[fi]()
                            fi += 1
                    for pj, ppt in pend:
                        pv_chunk(h, half, pj, ppt, yt)
                    finish(h, half, yt)
            # remaining proj: tt 8..15
            for tt in range(8, TT):
                for nn in range(2):
                    proj_chunk(tt, nn)

    nc.compile()
    return nc


def make_core_inputs(x, W_attn, b_attn, W_proj, n_cores=8, HC=8, D=64):
    """Host-side sharding: per-core input dicts."""
    B, T, C = x.shape
    CO = HC * D
    NP = CO // P
    in_maps = []
    for c in range(n_cores):
        b = c // (n_cores // B)
        h0 = (c % (n_cores // B)) * HC
        lo = h0 * D
        bq = b_attn[lo:lo + CO]
        bk = b_attn[C + lo:C + lo + CO]
        bv = b_attn[2 * C + lo:2 * C + lo + CO]
        bf = _bf16_np()
        in_maps.append({
            "xt": np.ascontiguousarray(x[b].T).astype(bf),
            "wq": np.ascontiguousarray(W_attn[:, lo:lo + CO]).astype(bf),
            "wk": np.ascontiguousarray(W_attn[:, C + lo:C + lo + CO]).astype(bf),
            "wv": np.ascontiguousarray(W_attn[:, 2 * C + lo:2 * C + lo + CO]).astype(bf),
            "bq": np.ascontiguousarray(bq.reshape(NP, P).T),
            "bk": np.ascontiguousarray(bk.reshape(NP, P).T),
            "bvb": np.tile(bv[None, :], (P, 1)),
            "ones": np.ones((P, (T // P) * HC), _bf16_np()),
            "wp": np.ascontiguousarray(W_proj[lo:lo + CO, :]),
        })
    return in_maps


_CACHE = {}


def _get_program():
    if "nc" not in _CACHE:
        _CACHE["nc"] = build_program()
    return _CACHE["nc"]


def run_on_cores(x, W_attn, b_attn, W_proj, b_proj, trace=False):
    """Returns (full output [B,T,C], BassKernelResults)."""
    from concourse.bass_utils import run_bass_kernel_spmd

    x = np.asarray(x, np.float32)
    W_attn = np.asarray(W_attn, np.float32)
    b_attn = np.asarray(b_attn, np.float32)
    W_proj = np.asarray(W_proj, np.float32)
    b_proj = np.asarray(b_proj, np.float32)

    nc = _get_program()
    in_maps = make_core_inputs(x, W_attn, b_attn, W_proj)
    res = run_bass_kernel_spmd(nc, in_maps, core_ids=list(range(8)), trace=trace)
    B, T, C = x.shape
    out = np.empty((B, T, C), np.float32)
    for b in range(B):
        out[b] = (res.results[2 * b]["out"].astype(np.float32)
                  + res.results[2 * b + 1]["out"].astype(np.float32)
                  + b_proj[None, :])
    return out, res


def kernel(x, W_attn, b_attn, W_proj, b_proj):
    out, _ = run_on_cores(x, W_attn, b_attn, W_proj, b_proj, trace=False)
    return out


# revision 11
# speedup vs baseline: 1.2173x; 1.2173x over previous
"""Causal self-attention (B=4, T=2048, C=1024, H=16) on 8 trn2 NeuronCores.

Sharding: core c -> batch b = c//2, heads h0 = (c%2)*8 .. h0+8 (tensor
parallel over heads: c_attn columns / c_proj rows split). Each core computes a
partial projection output [T, C] in bf16; the host sums the two partials per
batch and adds b_proj.

v2: single interleaved emission schedule so ScalarE exp overlaps all phases:
  - qkv projections chunked to [128,512] PSUM accumulators (1 bank each)
  - attention per (head, T/2-half): S^T superchunks [128,<=1024] -> exp ->
    PV accumulate into per-half yt [MV,1024] PSUM tile
  - v-stage / next-m qk-stage / proj chunks emitted as PE filler between
    attention chunks (keeps PE busy while exp runs, starts exp at ~25us)
  - output partials in bf16 (halves output DMA; rel-err ~4e-3 total)
"""

import numpy as np

P = 128


def _bf16_np():
    import ml_dtypes
    return ml_dtypes.bfloat16


def build_program(T=2048, C=1024, HC=8, D=64, num_devices=8, trn="TRN2"):
    import concourse.mybir as mybir
    import concourse.tile as tile
    from concourse import bacc
    from concourse.masks import make_upper_triangular

    W = 512          # matmul moving-dim chunk
    KC = C // P      # contraction tiles over C (8)
    CO = HC * D      # this core's qkv channel block (512)
    NP = CO // P     # head pairs per core (4)
    TT = T // P      # T tiles (16)
    T2 = T // 2      # half length (1024)
    NC512 = T // W   # 512-chunks across T (4)
    MV = 80          # PV stationary columns: D v-cols + 1 ones + pad
    dt32 = mybir.dt.float32
    f32r = mybir.dt.float32r
    bf16 = mybir.dt.bfloat16
    ActF = mybir.ActivationFunctionType
    Alu = mybir.AluOpType
    scale = 1.0 / float(np.sqrt(D))

    nc = bacc.Bacc(trn, target_bir_lowering=False, debug=False,
                   enable_asserts=False, num_devices=num_devices)

    xt_d = nc.dram_tensor("xt", [C, T], bf16, kind="ExternalInput")
    wq_d = nc.dram_tensor("wq", [C, CO], bf16, kind="ExternalInput")
    wk_d = nc.dram_tensor("wk", [C, CO], bf16, kind="ExternalInput")
    wv_d = nc.dram_tensor("wv", [C, CO], bf16, kind="ExternalInput")
    bq_d = nc.dram_tensor("bq", [P, NP], dt32, kind="ExternalInput")
    bk_d = nc.dram_tensor("bk", [P, NP], dt32, kind="ExternalInput")
    bvb_d = nc.dram_tensor("bvb", [P, CO], dt32, kind="ExternalInput")
    ones_d = nc.dram_tensor("ones", [P, TT * HC], bf16, kind="ExternalInput")
    wp_d = nc.dram_tensor("wp", [CO, C], f32r, kind="ExternalInput")
    out_d = nc.dram_tensor("out", [T, C], bf16, kind="ExternalOutput")
    lsc_d = nc.dram_tensor("lsc", [T], dt32)
    lsc2_d = nc.dram_tensor("lsc2", [T], dt32)

    with tile.TileContext(nc) as tc:
        with tc.tile_pool(name="const", bufs=1) as cpool, \
             tc.tile_pool(name="pers", bufs=1) as pers, \
             tc.tile_pool(name="wts", bufs=3 * KC) as wpool, \
             tc.tile_pool(name="wpj", bufs=1) as wppool, \
             tc.tile_pool(name="xtp", bufs=KC * NC512) as xpool, \
             tc.tile_pool(name="ptp", bufs=4) as ptpool, \
             tc.tile_pool(name="ysp", bufs=2) as yspool, \
             tc.tile_pool(name="nrm", bufs=2) as nrmpool, \
             tc.tile_pool(name="ost", bufs=3) as opool, \
             tc.tile_pool(name="psX", bufs=2, space="PSUM") as psX, \
             tc.tile_pool(name="psS", bufs=2, space="PSUM") as psS, \
             tc.tile_pool(name="psY", bufs=2, space="PSUM") as psY:

            # ---- constants, persistent tensors, prefetch DMAs ----
            tri = cpool.tile([P, P], bf16)
            make_upper_triangular(nc, tri[:], val=1.0, diag=True)
            bq_sb = cpool.tile([P, NP], dt32)
            nc.scalar.dma_start(bq_sb[:], bq_d.ap())
            bk_sb = cpool.tile([P, NP], dt32)
            nc.scalar.dma_start(bk_sb[:], bk_d.ap())
            bvb_sb = cpool.tile([P, CO], dt32)
            nc.scalar.dma_start(bvb_sb[:], bvb_d.ap())
            bvb_v = bvb_sb[:].rearrange("p (h d) -> p h d", d=D)

            qT = pers.tile([P, NP, T], bf16, tag="qT")
            kT = pers.tile([P, HC, T], bf16, tag="kT")
            vaug = pers.tile([P, TT, HC, MV], bf16, tag="vaug")
            yT = pers.tile([P, NP, T], f32r, tag="yT")
            nc.vector.memset(kT[:], 0.0)
            nc.vector.memset(vaug[:], 0.0)
            nc.scalar.dma_start(
                vaug[:, :, :, D],
                ones_d.ap().rearrange("p (a b) -> p a b", b=HC))

            xt_view = xt_d.ap().rearrange("(kc p) t -> kc p t", p=P)
            dmae = [nc.sync, nc.gpsimd]
            di = 0

            def dma(dst, src):
                nonlocal di
                dmae[di % 2].dma_start(dst, src)
                di += 1

            # DMA order: wq + x[c0] (first qk chunks), then wv (v_tt fillers
            # start in h0's window), then rest of x, then wk (needed at h1)
            wq_t = []
            xts = [[None] * NC512 for _ in range(KC)]
            wq_view = wq_d.ap().rearrange("(kc p) n -> kc p n", p=P)
            for kc in range(KC):
                wt = wpool.tile([P, CO], bf16, tag="w")
                dma(wt[:], wq_view[kc])
                wq_t.append(wt)
                xtc = xpool.tile([P, W], bf16, tag="xt")
                dma(xtc[:], xt_view[kc][:, 0:W])
                xts[kc][0] = xtc

            def load_w(w_d):
                view = w_d.ap().rearrange("(kc p) n -> kc p n", p=P)
                tiles = []
                for kc in range(KC):
                    wt = wpool.tile([P, CO], bf16, tag="w")
                    dma(wt[:], view[kc])
                    tiles.append(wt)
                return tiles

            wv_t = load_w(wv_d)
            for cg in range(1, NC512):
                for kc in range(KC):
                    xtc = xpool.tile([P, W], bf16, tag="xt")
                    dma(xtc[:], xt_view[kc][:, cg * W:(cg + 1) * W])
                    xts[kc][cg] = xtc
            wk_t = load_w(wk_d)
            wpsb = wppool.tile([P, NP, C], f32r)

            # ---- stage emitters (each a small closure; order = schedule) ----
            def q_chunk(m, c):
                ps = psX.tile([P, W], dt32, tag="x")
                for kc in range(KC):
                    nc.tensor.matmul(
                        ps[:], wq_t[kc][:, m * P:(m + 1) * P], xts[kc][c][:],
                        start=(kc == 0), stop=(kc == KC - 1),
                        skip_group_check=True)
                nc.scalar.activation(
                    qT[:, m, c * W:(c + 1) * W], ps[:],
                    ActF.Identity, bias=bq_sb[:, m:m + 1], scale=1.0)

            def k_chunk(m, c):
                ps = psX.tile([P, W], dt32, tag="x")
                for kc in range(KC):
                    nc.tensor.matmul(
                        ps[:], wk_t[kc][:, m * P:(m + 1) * P], xts[kc][c][:],
                        start=(kc == 0), stop=(kc == KC - 1),
                        skip_group_check=True)
                sl = slice(c * W, (c + 1) * W)
                nc.scalar.activation(
                    kT[0:D, 2 * m, sl], ps[0:D, :],
                    ActF.Identity, bias=bk_sb[0:D, m:m + 1], scale=1.0)
                nc.scalar.activation(
                    kT[D:P, 2 * m + 1, sl], ps[D:P, :],
                    ActF.Identity, bias=bk_sb[D:P, m:m + 1], scale=1.0)

            def v_tt(tt):
                ps = psX.tile([P, CO], dt32, tag="x")
                c, off = (tt * P) // W, (tt * P) % W
                for kc in range(KC):
                    nc.tensor.matmul(
                        ps[:], xts[kc][c][:, off:off + P], wv_t[kc][:],
                        start=(kc == 0), stop=(kc == KC - 1),
                        skip_group_check=True)
                nc.vector.scalar_tensor_tensor(
                    out=vaug[:, tt, :, 0:D],
                    in0=ps[:].rearrange("p (h d) -> p h d", d=D),
                    scalar=1.0, in1=bvb_v,
                    op0=Alu.mult, op1=Alu.add)

            def s_chunk(h, half, j):
                m = h // 2
                jb = j * P
                lo, hi = half * T2, half * T2 + T2
                q0 = max(jb, lo)
                L = hi - q0
                pt = ptpool.tile([P, L], bf16, tag="pt")
                sps = psS.tile([P, T2], dt32, tag="s")
                for qs in range(q0, hi, W):
                    w = min(W, hi - qs)
                    nc.tensor.matmul(
                        sps[:, qs - q0:qs - q0 + w],
                        kT[:, h, jb:jb + P],
                        qT[:, m, qs:qs + w],
                        start=True, stop=True, skip_group_check=True)
                nc.scalar.activation(pt[:], sps[:, 0:L], ActF.Exp, scale=scale)
                if q0 == jb:
                    nc.vector.tensor_mul(pt[:, 0:P], pt[:, 0:P], tri[:])
                return pt

            def pv_chunk(h, half, j, pt, yts):
                # W-grid chunks; each chunk lands in one quarter tile
                # yts[cg] ([MV, W], one PSUM bank each)
                jb = j * P
                lo, hi = half * T2, half * T2 + T2
                q0 = max(jb, lo)
                for cg in range(q0 // W, hi // W):
                    qs = max(W * cg, q0)
                    w = W * (cg + 1) - qs
                    last_j = (qs + w - 1) // P
                    yt = yts[cg - lo // W]
                    nc.tensor.matmul(
                        yt[:, qs - W * cg:qs - W * cg + w],
                        vaug[:, j, h, :],
                        pt[:, qs - q0:qs - q0 + w],
                        start=(j == 0), stop=(j == last_j),
                        skip_group_check=True)

            def finish(h, qq, yt):
                # normalize quarter qq (columns [qq*W, qq*W+W)) of head h
                m, r0 = h // 2, (h % 2) * D
                lo = qq * W
                ys = yspool.tile([D + 1, W], dt32, tag="ys")
                nc.vector.tensor_copy(ys[:], yt[0:D + 1, :])
                nc.sync.dma_start(
                    lsc_d.ap()[lo:lo + W].rearrange("(o t) -> o t", o=1),
                    ys[D:D + 1, :])
                l128 = nrmpool.tile([P, W // P], dt32, tag="l128")
                nc.gpsimd.dma_start(
                    l128[:],
                    lsc_d.ap()[lo:lo + W].rearrange("(p c) -> p c", p=P))
                nc.vector.reciprocal(l128[:], l128[:])
                nc.gpsimd.dma_start(
                    lsc2_d.ap()[lo:lo + W].rearrange("(p c) -> p c", p=P),
                    l128[:])
                bc = nrmpool.tile([D, W], dt32, tag="bc")
                nc.sync.dma_start(
                    bc[:],
                    lsc2_d.ap()[lo:lo + W].rearrange(
                        "(o t) -> o t", o=1).broadcast_to([D, W]))
                nc.vector.tensor_mul(
                    yT[r0:r0 + D, m, lo:lo + W], ys[0:D, :], bc[:])

            def proj_chunk(tt, nn):
                po = psX.tile([P, W], dt32, tag="x")
                for kt in range(NP):
                    nc.tensor.matmul(
                        po[:], yT[:, kt, tt * P:(tt + 1) * P],
                        wpsb[:, kt, nn * W:(nn + 1) * W],
                        start=(kt == 0), stop=(kt == NP - 1),
                        skip_group_check=True)
                ot = opool.tile([P, W], bf16, tag="ot")
                nc.vector.tensor_copy(ot[:], po[:])
                dma(out_d.ap()[tt * P:(tt + 1) * P, nn * W:(nn + 1) * W],
                    ot[:])

            # ---- the schedule ----
            # One global S->exp->PV pipeline crossing head/half boundaries
            # (no flush stalls). PV trails S by DEPTH chunks; each quarter
            # of a head's yt is finished (normalized) as soon as its last
            # PV lands, which staggers PSUM release and lets proj overlap
            # the final head. Fillers (v/qk/proj chunks) are paced by
            # cumulative exp-work so PE has work wherever exp is dense.
            def head_fillers(h):
                items = []
                if h == 0:
                    items = [lambda tt=tt: v_tt(tt) for tt in range(TT)]
                elif h in (2, 3, 4, 5):
                    mm = h // 2 + 1
                    cs = range(0, 2) if h % 2 == 0 else range(2, 4)
                    for c in cs:
                        items.append(lambda c=c: q_chunk(mm, c))
                        items.append(lambda c=c: k_chunk(mm, c))
                elif h == 6:
                    items = [lambda: nc.scalar.dma_start(
                        wpsb[:],
                        wp_d.ap().rearrange("(kt p) n -> p kt n", p=P))]
                elif h == 7:
                    # tt0-7 gated by all-q0/q1 finishes, tt8-11 by q2
                    items = [lambda t=t: proj_chunk(t // 2, t % 2)
                             for t in range(24)]
                return items

            # chunk list per head with exp-cost weights; fillers paced by
            # cumulative weight (except h0/h7 which need hard gating)
            def chunk_weights():
                ws = []
                for half in (0, 1):
                    for j in range((half * T2 + T2) // P):
                        ws.append(T2 - max(j * P - half * T2, 0))
                return ws  # 24 entries, sum = 17408

            WTS = chunk_weights()
            CUM = []
            s = 0
            for w_ in WTS:
                s += w_
                CUM.append(s)
            TOT = s

            def wanted_frac(h, i, n):
                if h == 0:
                    return min(i + 1, n)  # v(tt) by chunk tt (PV j=tt dep)
                if h == 7:
                    # proj tt0-7 gated by half0 finishes (all done by i=10
                    # when h7's q1 finish pops); tt8+ gated by q2 finish
                    if i < 10:
                        return 0
                    if i < 22:
                        return min(2 * (i - 9), 16)
                    return 16 + 6 * (i - 21)
                return (CUM[i] * n + TOT - 1) // TOT

            # qk for m=0 and m=1 up front (m1 consumed from h2's window on)
            for c in range(NC512):
                q_chunk(0, c)
                k_chunk(0, c)
            qk1 = [f for c in range(NC512)
                   for f in (lambda c=c: q_chunk(1, c),
                             lambda c=c: k_chunk(1, c))]

            DEPTH = 2
            pend = []
            ytq = {}     # (h, half) -> [quarter tiles]

            def pop_pv():
                ph, phalf, pj, ppt = pend.pop(0)
                key = (ph, phalf)
                if pj == 0:
                    ytq[key] = [psY.tile([MV, W], dt32, tag="yt", name="yt")
                                for _ in range(T2 // W)]
                pv_chunk(ph, phalf, pj, ppt, ytq[key])
                # finish any quarter whose last writer just landed
                for ql in range(T2 // W):
                    last_j = ((phalf * T2 + ql * W + W) // P) - 1
                    if pj == last_j:
                        finish(ph, phalf * 2 + ql, ytq[key][ql])

            for h in range(HC):
                fillers = head_fillers(h)
                if h == 1:
                    fillers = qk1  # m=1 weights during h1 (needed by h2)
                fi = 0
                i = 0
                for half in (0, 1):
                    for j in range((half * T2 + T2) // P):
                        while fi < len(fillers) and \
                                fi < wanted_frac(h, i, len(fillers)):
                            fillers[fi]()
                            fi += 1
                        pt = s_chunk(h, half, j)
                        pend.append((h, half, j, pt))
                        if len(pend) > DEPTH:
                            pop_pv()
                        i += 1
                while fi < len(fillers):
                    fillers[fi]()
                    fi += 1
            while pend:
                pop_pv()
            # remaining proj: tt 12..15 (needs last quarter finishes)
            for tt in range(12, TT):
                for nn in range(2):
                    proj_chunk(tt, nn)

    nc.compile()
    return nc


def make_core_inputs(x, W_attn, b_attn, W_proj, n_cores=8, HC=8, D=64):
    """Host-side sharding: per-core input dicts."""
    B, T, C = x.shape
    CO = HC * D
    NP = CO // P
    in_maps = []
    for c in range(n_cores):
        b = c // (n_cores // B)
        h0 = (c % (n_cores // B)) * HC
        lo = h0 * D
        bq = b_attn[lo:lo + CO]
        bk = b_attn[C + lo:C + lo + CO]
        bv = b_attn[2 * C + lo:2 * C + lo + CO]
        bf = _bf16_np()
        in_maps.append({
            "xt": np.ascontiguousarray(x[b].T).astype(bf),
            "wq": np.ascontiguousarray(W_attn[:, lo:lo + CO]).astype(bf),
            "wk": np.ascontiguousarray(W_attn[:, C + lo:C + lo + CO]).astype(bf),
            "wv": np.ascontiguousarray(W_attn[:, 2 * C + lo:2 * C + lo + CO]).astype(bf),
            "bq": np.ascontiguousarray(bq.reshape(NP, P).T),
            "bk": np.ascontiguousarray(bk.reshape(NP, P).T),
            "bvb": np.tile(bv[None, :], (P, 1)),
            "ones": np.ones((P, (T // P) * HC), _bf16_np()),
            "wp": np.ascontiguousarray(W_proj[lo:lo + CO, :]),
        })
    return in_maps


_CACHE = {}


def _get_program():
    if "nc" not in _CACHE:
        _CACHE["nc"] = build_program()
    return _CACHE["nc"]


def run_on_cores(x, W_attn, b_attn, W_proj, b_proj, trace=False):
    """Returns (full output [B,T,C], BassKernelResults)."""
    from concourse.bass_utils import run_bass_kernel_spmd

    x = np.asarray(x, np.float32)
    W_attn = np.asarray(W_attn, np.float32)
    b_attn = np.asarray(b_attn, np.float32)
    W_proj = np.asarray(W_proj, np.float32)
    b_proj = np.asarray(b_proj, np.float32)

    nc = _get_program()
    in_maps = make_core_inputs(x, W_attn, b_attn, W_proj)
    res = run_bass_kernel_spmd(nc, in_maps, core_ids=list(range(8)), trace=trace)
    B, T, C = x.shape
    out = np.empty((B, T, C), np.float32)
    for b in range(B):
        out[b] = (res.results[2 * b]["out"].astype(np.float32)
                  + res.results[2 * b + 1]["out"].astype(np.float32)
                  + b_proj[None, :])
    return out, res


def kernel(x, W_attn, b_attn, W_proj, b_proj):
    out, _ = run_on_cores(x, W_attn, b_attn, W_proj, b_proj, trace=False)
    return out
